# revision 2
# baseline (speedup 1.0000x reference)
"""Two-layer GRU encoder (B=64, T=12, N=325, D=2, H=256) on 8 TRN2 NeuronCores.

Strategy: data-parallel over batch (8 B-slices, one per core; per-core row
count M = 8*325 = 2600). Everything on-device uses a transposed
"feature-on-partition" layout: hidden state h is stored as (128, 2*m) bf16
tiles whose halves are feature chunks [0:128] and [128:256]; GRU weights sit
stationary in the PE as bf16 lhsT tiles and the batch dimension streams as
the matmul moving operand in chunks of 512 (PSUM bank limit).

Per step/chunk/layer: x-projection matmul (K=2 from raw x for layer 0,
K=256 from h0' for layer 1) accumulates with the recurrent matmul (K=256)
directly in PSUM; sigmoid/tanh run on the scalar engine with the combined
biases applied via the per-partition bias operand; r*h and the state blend
h' = h + z*(c - h) run on the vector engine in bf16 (2x mode).

The host wrapper shards/transposes inputs, runs the SPMD kernel via
run_bass_kernel_spmd on cores 0-7, and reassembles the (2, 64, 325, 256)
float32 output.
"""

import numpy as np
import ml_dtypes
from contextlib import ExitStack

import concourse.bass as bass
import concourse.tile as tile
from concourse import bacc, mybir
from concourse import bass_utils

BF16 = ml_dtypes.bfloat16
AF = mybir.ActivationFunctionType

H = 256
T = 12
B = 64
N = 325
D = 2
NCORES = 8
B_SH = B // NCORES            # 8
M = B_SH * N                  # 2600
CW = 512
CHUNKS = [(o, min(CW, M - o)) for o in range(0, M, CW)]
DT = mybir.dt

_CACHE = {}


def _emit_layer(nc, psum, work, t, mw, emit_xp, whzr_sb, whh_sb, bias_sb, bcol0,
                h_prev, h_new, uid):
    """Emit one GRU cell for one (timestep, m-chunk, layer).

    emit_xp(g, ptile, more): emits the x-projection matmuls into psum tile
    `ptile` for gate-feature chunk g (0..5 = za,zb,ra,rb,ca,cb); `more` is
    True when recurrent matmuls will accumulate on top afterwards.
    gate order in weight cols: z:[0:256] r:[256:512] c:[512:768].
    """
    first = t == 0
    f32 = DT.float32

    # ---- z/r pre-activations in PSUM ----
    pz, pr = [], []
    for gi, plist in ((2, pr), (0, pz)):  # r first: it's on the critical path
        for half in (0, 1):
            g = gi + half
            pt = psum.tile([128, mw], f32, tag="ps", name=f"p{uid}_g{g}")
            emit_xp(g, pt, more=not first)
            if not first:
                for k in (0, 1):
                    nc.tensor.matmul(
                        pt[:],
                        whzr_sb[:, k * 512 + g * 128: k * 512 + (g + 1) * 128],
                        h_prev[:, k * mw:(k + 1) * mw],
                        start=False, stop=(k == 1),
                    )
            plist.append(pt)

    s_r = work.tile([128, 2 * mw], DT.bfloat16, tag="sr", name=f"sr{uid}")
    for half in (0, 1):
        nc.scalar.activation(s_r[:, half * mw:(half + 1) * mw], pr[half][:],
                             AF.Sigmoid, bias=bias_sb[:, bcol0 + 2 + half: bcol0 + 3 + half])
    s_z = work.tile([128, 2 * mw], DT.bfloat16, tag="sz", name=f"sz{uid}")
    for half in (0, 1):
        nc.scalar.activation(s_z[:, half * mw:(half + 1) * mw], pz[half][:],
                             AF.Sigmoid, bias=bias_sb[:, bcol0 + half: bcol0 + 1 + half])

    # ---- candidate ----
    if not first:
        rh = work.tile([128, 2 * mw], DT.bfloat16, tag="rh", name=f"rh{uid}")
        nc.vector.tensor_mul(rh[:], s_r[:], h_prev[:])

    pc = []
    for half in (0, 1):
        g = 4 + half
        pt = psum.tile([128, mw], f32, tag="ps", name=f"p{uid}_g{g}")
        emit_xp(g, pt, more=not first)
        if not first:
            for k in (0, 1):
                nc.tensor.matmul(
                    pt[:],
                    whh_sb[:, k * 256 + half * 128: k * 256 + (half + 1) * 128],
                    rh[:, k * mw:(k + 1) * mw],
                    start=False, stop=(k == 1),
                )
        pc.append(pt)

    c = work.tile([128, 2 * mw], DT.bfloat16, tag="c", name=f"c{uid}")
    for half in (0, 1):
        nc.scalar.activation(c[:, half * mw:(half + 1) * mw], pc[half][:],
                             AF.Tanh, bias=bias_sb[:, bcol0 + 4 + half: bcol0 + 5 + half])

    # ---- state blend: h' = h + z*(c - h)   (h'=z*c when h==0 at t=0) ----
    if first:
        nc.vector.tensor_mul(h_new[:], s_z[:], c[:])
    else:
        d = work.tile([128, 2 * mw], DT.bfloat16, tag="d", name=f"d{uid}")
        nc.vector.tensor_sub(d[:], c[:], h_prev[:])
        zd = work.tile([128, 2 * mw], DT.bfloat16, tag="zd", name=f"zd{uid}")
        nc.vector.tensor_mul(zd[:], s_z[:], d[:])
        nc.vector.tensor_add(h_new[:], h_prev[:], zd[:])


def _build_nc():
    nc = bacc.Bacc("TRN2", target_bir_lowering=False, debug=False,
                   enable_asserts=False)
    bf = DT.bfloat16

    xt_d = nc.dram_tensor("xt", (D, T * M), bf, kind="ExternalInput").ap()
    wx0_d = nc.dram_tensor("wx0", (D, 768), bf, kind="ExternalInput").ap()
    whzr0_d = nc.dram_tensor("whzr0", (128, 1024), bf, kind="ExternalInput").ap()
    whh0_d = nc.dram_tensor("whh0", (128, 512), bf, kind="ExternalInput").ap()
    wx1_d = nc.dram_tensor("wx1", (128, 1536), bf, kind="ExternalInput").ap()
    whzr1_d = nc.dram_tensor("whzr1", (128, 1024), bf, kind="ExternalInput").ap()
    whh1_d = nc.dram_tensor("whh1", (128, 512), bf, kind="ExternalInput").ap()
    bias_d = nc.dram_tensor("bias", (128, 12), DT.float32, kind="ExternalInput").ap()
    out_d = nc.dram_tensor("out", (2, len(CHUNKS), 128, 2 * CW), bf,
                           kind="ExternalOutput").ap()

    with tile.TileContext(nc) as tc, ExitStack() as ctx:
        const = ctx.enter_context(tc.tile_pool(name="const", bufs=1))
        hpool = ctx.enter_context(tc.tile_pool(name="hstate", bufs=1))
        work = ctx.enter_context(tc.tile_pool(name="work", bufs=3))
        psum = ctx.enter_context(tc.tile_pool(name="psum", bufs=8, space="PSUM"))

        def load(name, dram, shape, dtype=bf):
            t_ = const.tile(list(shape), dtype, tag=name, name=name)
            nc.sync.dma_start(t_[:], dram[:])
            return t_

        xt_sb = load("xt", xt_d, (D, T * M))
        wx0_sb = load("wx0", wx0_d, (D, 768))
        whzr0_sb = load("whzr0", whzr0_d, (128, 1024))
        whh0_sb = load("whh0", whh0_d, (128, 512))
        wx1_sb = load("wx1", wx1_d, (128, 1536))
        whzr1_sb = load("whzr1", whzr1_d, (128, 1024))
        whh1_sb = load("whh1", whh1_d, (128, 512))
        bias_sb = load("bias", bias_d, (128, 12), DT.float32)

        hst = {}
        for L in (0, 1):
            for ci, (m0, mw) in enumerate(CHUNKS):
                for pp in (0, 1):
                    nm = f"h{L}_{ci}_{pp}"
                    hst[(L, ci, pp)] = hpool.tile([128, 2 * mw], bf, tag=nm, name=nm)

        for t in range(T):
            pp_w = t % 2
            pp_r = 1 - pp_w
            for ci, (m0, mw) in enumerate(CHUNKS):
                x_rhs = xt_sb[:, t * M + m0: t * M + m0 + mw]
                h0_new = hst[(0, ci, pp_w)]

                def xp0(g, pt, more):
                    nc.tensor.matmul(pt[:], wx0_sb[:, g * 128:(g + 1) * 128],
                                     x_rhs, start=True, stop=not more)

                def xp1(g, pt, more):
                    for k in (0, 1):
                        nc.tensor.matmul(
                            pt[:], wx1_sb[:, k * 768 + g * 128: k * 768 + (g + 1) * 128],
                            h0_new[:, k * mw:(k + 1) * mw],
                            start=(k == 0), stop=(k == 1) and not more)

                _emit_layer(nc, psum, work, t, mw, xp0, whzr0_sb, whh0_sb,
                            bias_sb, 0, hst[(0, ci, pp_r)], h0_new,
                            uid=f"L0t{t}c{ci}")
                _emit_layer(nc, psum, work, t, mw, xp1, whzr1_sb, whh1_sb,
                            bias_sb, 6, hst[(1, ci, pp_r)], hst[(1, ci, pp_w)],
                            uid=f"L1t{t}c{ci}")

        ppf = (T - 1) % 2
        for L in (0, 1):
            for ci, (m0, mw) in enumerate(CHUNKS):
                nc.sync.dma_start(out_d[L, ci, :, 0:2 * mw], hst[(L, ci, ppf)][:])

    nc.compile()
    return nc


def _prep_weights(inputs):
    def bf(x):
        return np.ascontiguousarray(np.asarray(x, np.float32), dtype=BF16)

    def kstack(w):  # (256, C) -> (128, 2*C) with [K0 | K1] on cols
        w = np.asarray(w, np.float32)
        return bf(np.concatenate([w[:128], w[128:]], axis=1))

    bias = np.zeros((128, 12), np.float32)
    for L, (bx, bhzr, bhh) in enumerate(
            [(inputs["bx0"], inputs["bhzr0"], inputs["bhh0"]),
             (inputs["bx1"], inputs["bhzr1"], inputs["bhh1"])]):
        bz = bx[:H] + bhzr[:H]
        br = bx[H:2 * H] + bhzr[H:2 * H]
        bc = bx[2 * H:] + bhh
        for gi, v in enumerate((bz, br, bc)):
            bias[:, L * 6 + 2 * gi] = v[:128]
            bias[:, L * 6 + 2 * gi + 1] = v[128:]

    return {
        "wx0": bf(inputs["Wx0"]),
        "whzr0": kstack(inputs["Whzr0"]),
        "whh0": kstack(inputs["Whh0"]),
        "wx1": kstack(inputs["Wx1"]),
        "whzr1": kstack(inputs["Whzr1"]),
        "whh1": kstack(inputs["Whh1"]),
        "bias": bias,
    }


def kernel(**inputs):
    X = np.asarray(inputs["X"], np.float32)
    shared = _prep_weights(inputs)

    if "nc" not in _CACHE:
        _CACHE["nc"] = _build_nc()
    nc = _CACHE["nc"]

    in_maps = []
    for c in range(NCORES):
        Xc = X[c * B_SH:(c + 1) * B_SH]                      # (8, T, N, D)
        xt = np.ascontiguousarray(Xc.transpose(3, 1, 0, 2)).reshape(D, T * M)
        m = dict(shared)
        m["xt"] = np.ascontiguousarray(xt, dtype=BF16)
        in_maps.append(m)
    _CACHE["in_maps"] = in_maps

    res = bass_utils.run_bass_kernel_spmd(nc, in_maps, core_ids=list(range(NCORES)))

    out = np.empty((2, B, N, H), np.float32)
    for c in range(NCORES):
        arr = np.asarray(res.results[c]["out"], dtype=np.float32)  # (2,6,128,1024)
        per_core = np.empty((2, M, H), np.float32)
        for ci, (m0, mw) in enumerate(CHUNKS):
            blk = arr[:, ci, :, :2 * mw].reshape(2, 128, 2, mw)
            # [l, p, k, j] -> feature k*128+p, row m0+j
            per_core[:, m0:m0 + mw, :] = blk.transpose(0, 3, 2, 1).reshape(2, mw, H)
        out[:, c * B_SH:(c + 1) * B_SH] = per_core.reshape(2, B_SH, N, H)
    return out


# revision 5
# speedup vs baseline: 1.4919x; 1.4919x over previous
"""Two-layer GRU encoder (B=64, T=12, N=325, D=2, H=256) on 8 TRN2 NeuronCores.

Strategy: data-parallel over batch (8 B-slices, one per core; per-core row
count M = 8*325 = 2600). Everything on-device uses a transposed
"feature-on-partition" layout: hidden state h is stored as (128, 2*m) bf16
tiles whose halves are feature chunks [0:128] and [128:256]; GRU weights sit
stationary in the PE as bf16 lhsT tiles and the batch dimension streams as
the matmul moving operand in chunks of 512 (PSUM bank limit).

Per step/chunk/layer: x-projection matmul (K=2 from raw x for layer 0,
K=256 from h0' for layer 1) accumulates with the recurrent matmul (K=256)
directly in PSUM; sigmoid/tanh run on the scalar engine with the combined
biases applied via the per-partition bias operand; r*h and the state blend
h' = h + z*(c - h) run on the vector engine in bf16 (2x mode).

The host wrapper shards/transposes inputs, runs the SPMD kernel via
run_bass_kernel_spmd on cores 0-7, and reassembles the (2, 64, 325, 256)
float32 output.
"""

import numpy as np
import ml_dtypes
from contextlib import ExitStack

import concourse.bass as bass
import concourse.tile as tile
from concourse import bacc, mybir
from concourse import bass_utils

BF16 = ml_dtypes.bfloat16
AF = mybir.ActivationFunctionType

H = 256
T = 12
B = 64
N = 325
D = 2
NCORES = 8
B_SH = B // NCORES            # 8
M = B_SH * N                  # 2600
CW = 512
CHUNKS = [(o, min(CW, M - o)) for o in range(0, M, CW)]
DT = mybir.dt

_CACHE = {}


def _emit_zr_stage(nc, psum, work, t, mw, emit_xp, whzr_sb, bias_sb, bcol0,
                   h_prev, uid):
    """Stage A of one GRU cell: z/r pre-activations (PSUM), sigmoids, r*h.

    emit_xp(g, ptile, more): emits the x-projection matmuls into psum tile
    `ptile` for gate-feature chunk g (0..5 = za,zb,ra,rb,ca,cb); `more` is
    True when recurrent matmuls will accumulate on top afterwards.
    gate order in weight cols: z:[0:256] r:[256:512] c:[512:768].
    Returns (s_z, rh) for stage B.
    """
    first = t == 0
    f32 = DT.float32

    pz, pr = [], []
    for gi, plist in ((2, pr), (0, pz)):  # r first: it's on the critical path
        for half in (0, 1):
            g = gi + half
            pt = psum.tile([128, mw], f32, tag="ps", name=f"p{uid}_g{g}")
            emit_xp(g, pt, more=not first)
            if not first:
                for k in (0, 1):
                    nc.tensor.matmul(
                        pt[:],
                        whzr_sb[:, k * 512 + g * 128: k * 512 + (g + 1) * 128],
                        h_prev[:, k * mw:(k + 1) * mw],
                        start=False, stop=(k == 1),
                    )
            plist.append(pt)

    s_r = work.tile([128, 2 * mw], DT.bfloat16, tag="sr", name=f"sr{uid}")
    for half in (0, 1):
        nc.scalar.activation(s_r[:, half * mw:(half + 1) * mw], pr[half][:],
                             AF.Sigmoid, bias=bias_sb[:, bcol0 + 2 + half: bcol0 + 3 + half])
    s_z = work.tile([128, 2 * mw], DT.bfloat16, tag="sz", name=f"sz{uid}")
    for half in (0, 1):
        nc.scalar.activation(s_z[:, half * mw:(half + 1) * mw], pz[half][:],
                             AF.Sigmoid, bias=bias_sb[:, bcol0 + half: bcol0 + 1 + half])

    rh = None
    if not first:
        rh = work.tile([128, 2 * mw], DT.bfloat16, tag="rh", name=f"rh{uid}")
        nc.vector.tensor_mul(rh[:], s_r[:], h_prev[:])
    return s_z, rh


def _emit_cand_stage(nc, psum, work, t, mw, emit_xp, whh_sb, bias_sb, bcol0,
                     h_prev, h_new, s_z, rh, uid):
    """Stage B: candidate matmuls + tanh + state blend h' = h + z*(c-h)."""
    first = t == 0
    f32 = DT.float32

    pc = []
    for half in (0, 1):
        g = 4 + half
        pt = psum.tile([128, mw], f32, tag="ps", name=f"p{uid}_g{g}")
        emit_xp(g, pt, more=not first)
        if not first:
            for k in (0, 1):
                nc.tensor.matmul(
                    pt[:],
                    whh_sb[:, k * 256 + half * 128: k * 256 + (half + 1) * 128],
                    rh[:, k * mw:(k + 1) * mw],
                    start=False, stop=(k == 1),
                )
        pc.append(pt)

    c = work.tile([128, 2 * mw], DT.bfloat16, tag="c", name=f"c{uid}")
    for half in (0, 1):
        nc.scalar.activation(c[:, half * mw:(half + 1) * mw], pc[half][:],
                             AF.Tanh, bias=bias_sb[:, bcol0 + 4 + half: bcol0 + 5 + half])

    if first:
        nc.vector.tensor_mul(h_new[:], s_z[:], c[:])
    else:
        d = work.tile([128, 2 * mw], DT.bfloat16, tag="d", name=f"d{uid}")
        nc.vector.tensor_sub(d[:], c[:], h_prev[:])
        zd = work.tile([128, 2 * mw], DT.bfloat16, tag="zd", name=f"zd{uid}")
        nc.vector.tensor_mul(zd[:], s_z[:], d[:])
        nc.vector.tensor_add(h_new[:], h_prev[:], zd[:])


def _build_nc():
    nc = bacc.Bacc("TRN2", target_bir_lowering=False, debug=False,
                   enable_asserts=False)
    bf = DT.bfloat16

    xt_d = nc.dram_tensor("xt", (D, T * M), bf, kind="ExternalInput").ap()
    wx0_d = nc.dram_tensor("wx0", (D, 768), bf, kind="ExternalInput").ap()
    whzr0_d = nc.dram_tensor("whzr0", (128, 1024), bf, kind="ExternalInput").ap()
    whh0_d = nc.dram_tensor("whh0", (128, 512), bf, kind="ExternalInput").ap()
    wx1_d = nc.dram_tensor("wx1", (128, 1536), bf, kind="ExternalInput").ap()
    whzr1_d = nc.dram_tensor("whzr1", (128, 1024), bf, kind="ExternalInput").ap()
    whh1_d = nc.dram_tensor("whh1", (128, 512), bf, kind="ExternalInput").ap()
    bias_d = nc.dram_tensor("bias", (128, 12), DT.float32, kind="ExternalInput").ap()
    out_d = nc.dram_tensor("out", (2, len(CHUNKS), 128, 2 * CW), bf,
                           kind="ExternalOutput").ap()

    with tile.TileContext(nc) as tc, ExitStack() as ctx:
        const = ctx.enter_context(tc.tile_pool(name="const", bufs=1))
        hpool = ctx.enter_context(tc.tile_pool(name="hstate", bufs=1))
        work = ctx.enter_context(tc.tile_pool(name="work", bufs=4))
        psum = ctx.enter_context(tc.tile_pool(name="psum", bufs=8, space="PSUM"))

        def load(name, dram, shape, dtype=bf):
            t_ = const.tile(list(shape), dtype, tag=name, name=name)
            nc.sync.dma_start(t_[:], dram[:])
            return t_

        xt_sb = load("xt", xt_d, (D, T * M))
        wx0_sb = load("wx0", wx0_d, (D, 768))
        whzr0_sb = load("whzr0", whzr0_d, (128, 1024))
        whh0_sb = load("whh0", whh0_d, (128, 512))
        wx1_sb = load("wx1", wx1_d, (128, 1536))
        whzr1_sb = load("whzr1", whzr1_d, (128, 1024))
        whh1_sb = load("whh1", whh1_d, (128, 512))
        bias_sb = load("bias", bias_d, (128, 12), DT.float32)

        hst = {}
        for L in (0, 1):
            for ci, (m0, mw) in enumerate(CHUNKS):
                for pp in (0, 1):
                    nm = f"h{L}_{ci}_{pp}"
                    hst[(L, ci, pp)] = hpool.tile([128, 2 * mw], bf, tag=nm, name=nm)

        NCH = len(CHUNKS)

        def make_xp0(t, ci):
            m0, mw = CHUNKS[ci]
            x_rhs = xt_sb[:, t * M + m0: t * M + m0 + mw]

            def xp0(g, pt, more):
                nc.tensor.matmul(pt[:], wx0_sb[:, g * 128:(g + 1) * 128],
                                 x_rhs, start=True, stop=not more)
            return xp0

        def make_xp1(t, ci):
            mw = CHUNKS[ci][1]
            h0_new = hst[(0, ci, t % 2)]

            def xp1(g, pt, more):
                for k in (0, 1):
                    nc.tensor.matmul(
                        pt[:], wx1_sb[:, k * 768 + g * 128: k * 768 + (g + 1) * 128],
                        h0_new[:, k * mw:(k + 1) * mw],
                        start=(k == 0), stop=(k == 1) and not more)
            return xp1

        for t in range(T):
            pp_w = t % 2
            pp_r = 1 - pp_w
            for L, make_xp, whzr_sb, whh_sb, bcol0 in (
                    (0, make_xp0, whzr0_sb, whh0_sb, 0),
                    (1, make_xp1, whzr1_sb, whh1_sb, 6)):
                # Software-pipelined emission: the candidate stage of chunk
                # ci-1 is emitted after the z/r stage of chunk ci, so the PE
                # instruction stream never stalls on the sigmoid -> r*h chain
                # of the chunk it just fed.
                stage_a = {}
                for ci in range(NCH + 1):
                    if ci < NCH:
                        mw = CHUNKS[ci][1]
                        uid = f"L{L}t{t}c{ci}"
                        stage_a[ci] = _emit_zr_stage(
                            nc, psum, work, t, mw, make_xp(t, ci), whzr_sb,
                            bias_sb, bcol0, hst[(L, ci, pp_r)], uid)
                    if ci >= 1:
                        cj = ci - 1
                        mw = CHUNKS[cj][1]
                        uid = f"L{L}t{t}c{cj}"
                        s_z, rh = stage_a.pop(cj)
                        _emit_cand_stage(
                            nc, psum, work, t, mw, make_xp(t, cj), whh_sb,
                            bias_sb, bcol0, hst[(L, cj, pp_r)],
                            hst[(L, cj, pp_w)], s_z, rh, uid)

        ppf = (T - 1) % 2
        for L in (0, 1):
            for ci, (m0, mw) in enumerate(CHUNKS):
                nc.sync.dma_start(out_d[L, ci, :, 0:2 * mw], hst[(L, ci, ppf)][:])

    nc.compile()
    return nc


def _prep_weights(inputs):
    def bf(x):
        return np.ascontiguousarray(np.asarray(x, np.float32), dtype=BF16)

    def kstack(w):  # (256, C) -> (128, 2*C) with [K0 | K1] on cols
        w = np.asarray(w, np.float32)
        return bf(np.concatenate([w[:128], w[128:]], axis=1))

    bias = np.zeros((128, 12), np.float32)
    for L, (bx, bhzr, bhh) in enumerate(
            [(inputs["bx0"], inputs["bhzr0"], inputs["bhh0"]),
             (inputs["bx1"], inputs["bhzr1"], inputs["bhh1"])]):
        bz = bx[:H] + bhzr[:H]
        br = bx[H:2 * H] + bhzr[H:2 * H]
        bc = bx[2 * H:] + bhh
        for gi, v in enumerate((bz, br, bc)):
            bias[:, L * 6 + 2 * gi] = v[:128]
            bias[:, L * 6 + 2 * gi + 1] = v[128:]

    return {
        "wx0": bf(inputs["Wx0"]),
        "whzr0": kstack(inputs["Whzr0"]),
        "whh0": kstack(inputs["Whh0"]),
        "wx1": kstack(inputs["Wx1"]),
        "whzr1": kstack(inputs["Whzr1"]),
        "whh1": kstack(inputs["Whh1"]),
        "bias": bias,
    }


def kernel(**inputs):
    X = np.asarray(inputs["X"], np.float32)
    shared = _prep_weights(inputs)

    if "nc" not in _CACHE:
        _CACHE["nc"] = _build_nc()
    nc = _CACHE["nc"]

    in_maps = []
    for c in range(NCORES):
        Xc = X[c * B_SH:(c + 1) * B_SH]                      # (8, T, N, D)
        xt = np.ascontiguousarray(Xc.transpose(3, 1, 0, 2)).reshape(D, T * M)
        m = dict(shared)
        m["xt"] = np.ascontiguousarray(xt, dtype=BF16)
        in_maps.append(m)
    _CACHE["in_maps"] = in_maps

    res = bass_utils.run_bass_kernel_spmd(nc, in_maps, core_ids=list(range(NCORES)))

    out = np.empty((2, B, N, H), np.float32)
    for c in range(NCORES):
        arr = np.asarray(res.results[c]["out"], dtype=np.float32)  # (2,6,128,1024)
        per_core = np.empty((2, M, H), np.float32)
        for ci, (m0, mw) in enumerate(CHUNKS):
            blk = arr[:, ci, :, :2 * mw].reshape(2, 128, 2, mw)
            # [l, p, k, j] -> feature k*128+p, row m0+j
            per_core[:, m0:m0 + mw, :] = blk.transpose(0, 3, 2, 1).reshape(2, mw, H)
        out[:, c * B_SH:(c + 1) * B_SH] = per_core.reshape(2, B_SH, N, H)
    return out


# revision 17
# speedup vs baseline: 1.5639x; 1.0482x over previous
"""Two-layer GRU encoder (B=64, T=12, N=325, D=2, H=256) on 8 TRN2 NeuronCores.

Strategy: data-parallel over batch (8 B-slices, one per core; per-core row
count M = 8*325 = 2600). Everything on-device uses a transposed
"feature-on-partition" layout: hidden state h is stored as (128, 2*m) bf16
tiles whose halves are feature chunks [0:128] and [128:256]; GRU weights sit
stationary in the PE as bf16 lhsT tiles and the batch dimension streams as
the matmul moving operand in chunks of 512 (PSUM bank limit).

Per step/chunk/layer: x-projection matmul (K=2 from raw x for layer 0,
K=256 from h0' for layer 1) accumulates with the recurrent matmul (K=256)
directly in PSUM; sigmoid/tanh run on the scalar engine with the combined
biases applied via the per-partition bias operand; r*h and the state blend
h' = h + z*(c - h) run on the vector engine in bf16 (2x mode).

The host wrapper shards/transposes inputs, runs the SPMD kernel via
run_bass_kernel_spmd on cores 0-7, and reassembles the (2, 64, 325, 256)
float32 output.
"""

import numpy as np
import ml_dtypes
from contextlib import ExitStack

import concourse.bass as bass
import concourse.tile as tile
from concourse import bacc, mybir
from concourse import bass_utils

BF16 = ml_dtypes.bfloat16
AF = mybir.ActivationFunctionType

H = 256
T = 12
B = 64
N = 325
D = 2
NCORES = 8
B_SH = B // NCORES            # 8
M = B_SH * N                  # 2600
_CWS = [434, 434, 434, 434, 432, 432]   # even, <=512 (PSUM bank), sum = 2600
CHUNKS = []
_o = 0
for _w in _CWS:
    CHUNKS.append((_o, _w))
    _o += _w
CWMAX = max(_CWS)
DT = mybir.dt

_CACHE = {}


def _emit_zr_stage(nc, psum, work, t, mw, emit_xp, whzr_sb, bias_sb, bcol0,
                   h_prev, uid):
    """Stage A of one GRU cell: z/r pre-activations (PSUM), sigmoids, r*h.

    emit_xp(g, out_ap, more): emits the x-projection matmuls into psum slice
    `out_ap` for gate-feature chunk g (0..5 = za,zb,ra,rb,ca,cb); `more` is
    True when recurrent matmuls will accumulate on top afterwards.
    gate order in weight cols: z:[0:256] r:[256:512] c:[512:768].
    bcol0 is None when the layer's biases already rode the x-projection
    matmul (ones-row trick) — the sigmoids then fuse both feature halves.
    Returns (s_z, rh) for stage B.
    """
    first = t == 0
    f32 = DT.float32

    pz_pr = []
    for gi in (2, 0):  # r first: it's on the critical path
        pt = psum.tile([128, 2 * mw], f32, tag="ps", name=f"p{uid}_g{gi}")
        for half in (0, 1):
            g = gi + half
            sl = pt[:, half * mw:(half + 1) * mw]
            emit_xp(g, sl, more=not first)
            if not first:
                for k in (0, 1):
                    nc.tensor.matmul(
                        sl,
                        whzr_sb[:, k * 512 + g * 128: k * 512 + (g + 1) * 128],
                        h_prev[:, k * mw:(k + 1) * mw],
                        start=False, stop=(k == 1),
                    )
        pz_pr.append(pt)
    pr, pz = pz_pr

    s_r = work.tile([128, 2 * mw], DT.bfloat16, tag="sr", name=f"sr{uid}")
    s_z = work.tile([128, 2 * mw], DT.bfloat16, tag="sz", name=f"sz{uid}")
    if bcol0 is None:
        nc.scalar.activation(s_r[:], pr[:], AF.Sigmoid)
        nc.scalar.activation(s_z[:], pz[:], AF.Sigmoid)
    else:
        for half in (0, 1):
            nc.scalar.activation(s_r[:, half * mw:(half + 1) * mw],
                                 pr[:, half * mw:(half + 1) * mw], AF.Sigmoid,
                                 bias=bias_sb[:, bcol0 + 2 + half: bcol0 + 3 + half])
        for half in (0, 1):
            nc.scalar.activation(s_z[:, half * mw:(half + 1) * mw],
                                 pz[:, half * mw:(half + 1) * mw], AF.Sigmoid,
                                 bias=bias_sb[:, bcol0 + half: bcol0 + 1 + half])

    rh = None
    if not first:
        rh = work.tile([128, 2 * mw], DT.bfloat16, tag="rh", name=f"rh{uid}")
        nc.vector.tensor_mul(rh[:], s_r[:], h_prev[:])
    return s_z, rh


def _emit_cand_stage(nc, psum, work, t, mw, emit_xp, whh_sb, bias_sb, bcol0,
                     h_prev, h_new, s_z, rh, uid):
    """Stage B: candidate matmuls + tanh + state blend h' = h + z*(c-h)."""
    first = t == 0
    f32 = DT.float32

    pc = psum.tile([128, 2 * mw], f32, tag="ps", name=f"p{uid}_c")
    for half in (0, 1):
        g = 4 + half
        sl = pc[:, half * mw:(half + 1) * mw]
        emit_xp(g, sl, more=not first)
        if not first:
            for k in (0, 1):
                nc.tensor.matmul(
                    sl,
                    whh_sb[:, k * 256 + half * 128: k * 256 + (half + 1) * 128],
                    rh[:, k * mw:(k + 1) * mw],
                    start=False, stop=(k == 1),
                )

    c = work.tile([128, 2 * mw], DT.bfloat16, tag="c", name=f"c{uid}")
    if bcol0 is None:
        nc.scalar.activation(c[:], pc[:], AF.Tanh)
    else:
        for half in (0, 1):
            nc.scalar.activation(c[:, half * mw:(half + 1) * mw],
                                 pc[:, half * mw:(half + 1) * mw], AF.Tanh,
                                 bias=bias_sb[:, bcol0 + 4 + half: bcol0 + 5 + half])

    if first:
        nc.vector.tensor_mul(h_new[:], s_z[:], c[:])
    else:
        d = work.tile([128, 2 * mw], DT.bfloat16, tag="d", name=f"d{uid}")
        nc.vector.tensor_sub(d[:], c[:], h_prev[:])
        zd = work.tile([128, 2 * mw], DT.bfloat16, tag="zd", name=f"zd{uid}")
        nc.vector.tensor_mul(zd[:], s_z[:], d[:])
        nc.vector.tensor_add(h_new[:], h_prev[:], zd[:])


def _build_nc():
    nc = bacc.Bacc("TRN2", target_bir_lowering=False, debug=False,
                   enable_asserts=False)
    bf = DT.bfloat16

    xt_d = nc.dram_tensor("xt", (D + 1, T * M), bf, kind="ExternalInput").ap()
    wx0_d = nc.dram_tensor("wx0", (D + 1, 768), bf, kind="ExternalInput").ap()
    whzr0_d = nc.dram_tensor("whzr0", (128, 1024), bf, kind="ExternalInput").ap()
    whh0_d = nc.dram_tensor("whh0", (128, 512), bf, kind="ExternalInput").ap()
    wx1_d = nc.dram_tensor("wx1", (128, 1536), bf, kind="ExternalInput").ap()
    whzr1_d = nc.dram_tensor("whzr1", (128, 1024), bf, kind="ExternalInput").ap()
    whh1_d = nc.dram_tensor("whh1", (128, 512), bf, kind="ExternalInput").ap()
    bias_d = nc.dram_tensor("bias", (128, 12), DT.float32, kind="ExternalInput").ap()
    out_d = nc.dram_tensor("out", (2, len(CHUNKS), 128, 2 * CWMAX), bf,
                           kind="ExternalOutput").ap()

    with tile.TileContext(nc) as tc, ExitStack() as ctx:
        const = ctx.enter_context(tc.tile_pool(name="const", bufs=1))
        hpool = ctx.enter_context(tc.tile_pool(name="hstate", bufs=1))
        work = ctx.enter_context(tc.tile_pool(name="work", bufs=4))
        psum = ctx.enter_context(tc.tile_pool(name="psum", bufs=4, space="PSUM"))

        def load(name, dram, shape, dtype=bf):
            t_ = const.tile(list(shape), dtype, tag=name, name=name)
            nc.sync.dma_start(t_[:], dram[:])
            return t_

        xt_sb = load("xt", xt_d, (D + 1, T * M))
        wx0_sb = load("wx0", wx0_d, (D + 1, 768))
        whzr0_sb = load("whzr0", whzr0_d, (128, 1024))
        whh0_sb = load("whh0", whh0_d, (128, 512))
        wx1_sb = load("wx1", wx1_d, (128, 1536))
        whzr1_sb = load("whzr1", whzr1_d, (128, 1024))
        whh1_sb = load("whh1", whh1_d, (128, 512))
        bias_sb = load("bias", bias_d, (128, 12), DT.float32)

        hst = {}
        for L in (0, 1):
            for ci, (m0, mw) in enumerate(CHUNKS):
                for pp in (0, 1):
                    nm = f"h{L}_{ci}_{pp}"
                    hst[(L, ci, pp)] = hpool.tile([128, 2 * mw], bf, tag=nm, name=nm)

        NCH = len(CHUNKS)

        def make_xp0(t, ci):
            m0, mw = CHUNKS[ci]
            x_rhs = xt_sb[:, t * M + m0: t * M + m0 + mw]

            def xp0(g, out_ap, more):
                nc.tensor.matmul(out_ap, wx0_sb[:, g * 128:(g + 1) * 128],
                                 x_rhs, start=True, stop=not more)
            return xp0

        def make_xp1(t, ci):
            mw = CHUNKS[ci][1]
            h0_new = hst[(0, ci, t % 2)]

            def xp1(g, out_ap, more):
                for k in (0, 1):
                    nc.tensor.matmul(
                        out_ap, wx1_sb[:, k * 768 + g * 128: k * 768 + (g + 1) * 128],
                        h0_new[:, k * mw:(k + 1) * mw],
                        start=(k == 0), stop=(k == 1) and not more)
            return xp1

        for t in range(T):
            pp_w = t % 2
            pp_r = 1 - pp_w
            for L, make_xp, whzr_sb, whh_sb, bcol0 in (
                    (0, make_xp0, whzr0_sb, whh0_sb, None),
                    (1, make_xp1, whzr1_sb, whh1_sb, 6)):
                # Software-pipelined emission: the candidate stage of chunk
                # ci-1 is emitted after the z/r stage of chunk ci, so the PE
                # instruction stream never stalls on the sigmoid -> r*h chain
                # of the chunk it just fed.
                stage_a = {}
                for ci in range(NCH + 1):
                    if ci < NCH:
                        mw = CHUNKS[ci][1]
                        uid = f"L{L}t{t}c{ci}"
                        stage_a[ci] = _emit_zr_stage(
                            nc, psum, work, t, mw, make_xp(t, ci), whzr_sb,
                            bias_sb, bcol0, hst[(L, ci, pp_r)], uid)
                    if ci >= 1:
                        cj = ci - 1
                        mw = CHUNKS[cj][1]
                        uid = f"L{L}t{t}c{cj}"
                        s_z, rh = stage_a.pop(cj)
                        _emit_cand_stage(
                            nc, psum, work, t, mw, make_xp(t, cj), whh_sb,
                            bias_sb, bcol0, hst[(L, cj, pp_r)],
                            hst[(L, cj, pp_w)], s_z, rh, uid)

        ppf = (T - 1) % 2
        for L in (0, 1):
            for ci, (m0, mw) in enumerate(CHUNKS):
                nc.sync.dma_start(out_d[L, ci, :, 0:2 * mw], hst[(L, ci, ppf)][:])

    nc.compile()
    return nc


def _prep_weights(inputs):
    def bf(x):
        return np.ascontiguousarray(np.asarray(x, np.float32), dtype=BF16)

    def kstack(w):  # (256, C) -> (128, 2*C) with [K0 | K1] on cols
        w = np.asarray(w, np.float32)
        return bf(np.concatenate([w[:128], w[128:]], axis=1))

    bias = np.zeros((128, 12), np.float32)
    ball = {}
    for L, (bx, bhzr, bhh) in enumerate(
            [(inputs["bx0"], inputs["bhzr0"], inputs["bhh0"]),
             (inputs["bx1"], inputs["bhzr1"], inputs["bhh1"])]):
        bz = bx[:H] + bhzr[:H]
        br = bx[H:2 * H] + bhzr[H:2 * H]
        bc = bx[2 * H:] + bhh
        ball[L] = np.concatenate([bz, br, bc])
        for gi, v in enumerate((bz, br, bc)):
            bias[:, L * 6 + 2 * gi] = v[:128]
            bias[:, L * 6 + 2 * gi + 1] = v[128:]

    # layer 0 biases ride the x-projection matmul as a third lhsT row
    # (the matching rhs row is all-ones)
    wx0 = np.concatenate([np.asarray(inputs["Wx0"], np.float32),
                          ball[0][None, :]], axis=0)
    return {
        "wx0": bf(wx0),
        "whzr0": kstack(inputs["Whzr0"]),
        "whh0": kstack(inputs["Whh0"]),
        "wx1": kstack(inputs["Wx1"]),
        "whzr1": kstack(inputs["Whzr1"]),
        "whh1": kstack(inputs["Whh1"]),
        "bias": bias,
    }


def kernel(**inputs):
    X = np.asarray(inputs["X"], np.float32)
    shared = _prep_weights(inputs)

    if "nc" not in _CACHE:
        _CACHE["nc"] = _build_nc()
    nc = _CACHE["nc"]

    in_maps = []
    ones = np.ones((1, T * M), np.float32)
    for c in range(NCORES):
        Xc = X[c * B_SH:(c + 1) * B_SH]                      # (8, T, N, D)
        xt = np.ascontiguousarray(Xc.transpose(3, 1, 0, 2)).reshape(D, T * M)
        m = dict(shared)
        m["xt"] = np.ascontiguousarray(np.concatenate([xt, ones], axis=0),
                                       dtype=BF16)
        in_maps.append(m)
    _CACHE["in_maps"] = in_maps

    res = bass_utils.run_bass_kernel_spmd(nc, in_maps, core_ids=list(range(NCORES)))

    out = np.empty((2, B, N, H), np.float32)
    for c in range(NCORES):
        arr = np.asarray(res.results[c]["out"], dtype=np.float32)  # (2,6,128,2*CWMAX)
        per_core = np.empty((2, M, H), np.float32)
        for ci, (m0, mw) in enumerate(CHUNKS):
            blk = arr[:, ci, :, :2 * mw].reshape(2, 128, 2, mw)
            # [l, p, k, j] -> feature k*128+p, row m0+j
            per_core[:, m0:m0 + mw, :] = blk.transpose(0, 3, 2, 1).reshape(2, mw, H)
        out[:, c * B_SH:(c + 1) * B_SH] = per_core.reshape(2, B_SH, N, H)
    return out
